# revision 17
# baseline (speedup 1.0000x reference)
"""Single-head attention kernel for Trainium2, SPMD over 8 NeuronCores.

Problem: out = softmax((q@Wq+bq) @ (k@Wk+bk)^T / sqrt(768)) @ (v@Wv+bv)
Shapes: q,k,v [8, 2048, 768] fp32; W* [768, 64]; b* [64].

Strategy (v7): data-parallel over batch (1 batch per core). The kernel is a
software-pipelined stream bounded by the ScalarE exp wall (4.2M exps ~30us)
and the PE stream (~38us), with the DMA feed scheduled by need-time:

  - DMA facts (measured): ~400GB/s aggregate across 16 shared packet
    engines as long as per-partition rows are >=2-3KB; each HWDGE queue
    drains its transfers in order; the ~10-deep DMA semaphore pool blocks
    the ISSUING ENGINE when too many DMAs are outstanding.  Hence: the
    Scalar engine issues exactly 3 gate DMAs (wq/wk pack + q cols 0:512)
    and then runs only exps; the Sync engine issues everything else in
    need-time order (the sem-pool waits self-pace it).
  - q/k live in DRAM as 8 chunks of [128, 6x256] (s-chunk-major, e-chunks
    packed per row, 3KB rows); loads grab 2 chunks per issue.  v arrives as
    2 chunks of [128, 6x1024]; k chunk-pair c == t-block pairs 2c,2c+1.
  - A few dummy matmuls on the weight tile warm the PE HAM clock gate.
  - projections: x@W via W.T-as-lhsT accumulated over 6 e-chunks. wq/wk are
    duplicated [768,128] so qiT/kiT hold 2 identical copies across the 128
    partitions (feeds both row-tile groups).
  - scores in (pair, j) units: pair p = t-blocks (2p, 2p+1), j = 512
    q-columns. Two K=64 matmuls run CONCURRENTLY in PE row groups 0-63 /
    64-127 (tile_position row tiling), filling the two halves of one
    [128,1024] PSUM tile; a single FD-1024 exp on ScalarE (scale 1/sqrt(768)
    fused; scaled scores are N(0,1/12) so no max-subtraction needed).
  - v: viT = Wv.T @ vT, then PE transpose-mode matmuls against the identity
    (in the PE stream, so no cross-engine queue blocking), DVE-copied into
    vaug = [vi | ones] blocks (the ones make PSUM rows 64-127 of the output
    accumulate the softmax denominator for free).
  - output: per (t-block, j) matmul into a persistent [128,2048] PSUM
    region, woven into the score stream with a lag so the in-order PE queue
    never blocks on late operands.
  - normalize per 512-column chunk as soon as its accumulation stops (copy
    denom, reciprocal_approx_fast, multiply, DMA out).
"""

import numpy as np
from contextlib import ExitStack

import concourse.bass as bass
import concourse.mybir as mybir
import concourse.tile as tile
from concourse import bacc
from concourse.bass_utils import run_bass_kernel_spmd

E = 768  # n_embd
H = 64  # head size
S = 2048  # sequence length
B = 8  # batch == n_cores
EC = E // 128  # e chunks
TB = S // 128  # t blocks
NP = TB // 2  # t-block pairs
INV_SQRT_C = float(1.0 / np.sqrt(np.float32(E)))
WQK = EC * 128  # 768 packed weight cols per q/k tensor
WV = EC * H  # 384 packed weight cols for v
CW = 256  # q/k DRAM chunk width
NCH = S // CW  # 8 chunks

F16 = mybir.dt.float16
F32 = mybir.dt.float32

_CACHE = {}


def build_program():
    nc = bacc.Bacc(
        "TRN2",
        target_bir_lowering=False,
        debug=False,
        enable_asserts=False,
        num_devices=B,
    )

    # chunk-major host layouts: per-partition rows are >=3KB contiguous
    qT_d = nc.dram_tensor("qTc", [NCH * 128, EC * CW], F16, kind="ExternalInput")
    kT_d = nc.dram_tensor("kTc", [NCH * 128, EC * CW], F16, kind="ExternalInput")
    vT_d = nc.dram_tensor("vTc", [4 * 128, EC * 512], F16, kind="ExternalInput")
    w_d = nc.dram_tensor("wpack", [128, 2 * WQK + WV + H], F16, kind="ExternalInput")
    b_d = nc.dram_tensor("bpack", [128, 4], F32, kind="ExternalInput")
    outT_d = nc.dram_tensor("outT", [H, S], F32, kind="ExternalOutput")

    with tile.TileContext(nc) as tc, ExitStack() as ctx:
        const = ctx.enter_context(tc.tile_pool(name="const", bufs=1))
        xin = ctx.enter_context(tc.tile_pool(name="xin", bufs=1))
        acts = ctx.enter_context(tc.tile_pool(name="acts", bufs=1))

        w_t = const.tile([128, 2 * WQK + WV + H], F16, tag="w")
        b_t = const.tile([128, 4], F32, tag="b")
        warm = const.tile([128, 8], F32, tag="warm")
        wq_t = w_t[:, 0:WQK]
        wk_t = w_t[:, WQK : 2 * WQK]
        wv_t = w_t[:, 2 * WQK : 2 * WQK + WV]
        id_t = w_t[0:H, 2 * WQK + WV : 2 * WQK + WV + H]
        bq_t = b_t[:, 0:1]
        bk_t = b_t[:, 1:2]
        bv_t = b_t[0:H, 2:3]

        q_in = xin.tile([128, S * EC], F16, tag="q_in")
        k_in = xin.tile([128, S * EC], F16, tag="k_in")
        v_in = xin.tile([128, 2 * EC * 1024], F16, tag="v_in")

        qiT = acts.tile([128, S], F16, tag="qiT")
        kiT = acts.tile([128, S], F16, tag="kiT")
        viT = acts.tile([H, S], F16, tag="viT")
        vaug = acts.tile([128, TB * 128], F16, tag="vaug")
        att = acts.tile([128, NP * 4096], F16, tag="att")
        dsb = acts.tile([H, S], F32, tag="dsb")
        rec = acts.tile([H, S], F32, tag="rec")
        out_sb = acts.tile([H, S], F32, tag="out_sb")

        def x_cols(x_t, c0, w):
            # [128, nch, EC, CW] view of s-columns [c0, c0+w)
            nch = w // CW
            base = (c0 // CW) * EC * CW
            return x_t[:, base : base + nch * EC * CW].rearrange(
                "p (c e w) -> p c e w", c=nch, w=CW
            )

        def v_chunk(c):
            # [128, EC, 512] view of v s-columns [512c, 512c+512)
            return v_in[:, c * EC * 512 : (c + 1) * EC * 512].rearrange(
                "p (e w) -> p e w", w=512
            )

        def load_qk(eng, x_t, x_d, c, n):
            # n consecutive 256-col chunks in one issue (3KB packets)
            eng.dma_start(
                x_t[:, c * EC * CW : (c + n) * EC * CW].rearrange(
                    "p (c w) -> p c w", c=n
                ),
                x_d[c * 128 : (c + n) * 128, :].rearrange("(c p) w -> p c w", p=128),
            )

        def load_v(eng, c):
            eng.dma_start(
                v_in[:, c * EC * 512 : (c + 1) * EC * 512],
                vT_d[c * 128 : (c + 1) * 128, :],
            )

        # ---- Scalar HWDGE queue: just the 3-DMA gate, then exp warm ----
        nc.vector.memset(warm[:], 0.0)
        load_qk(nc.scalar, q_in, qT_d, 0, 1)
        load_qk(nc.scalar, q_in, qT_d, 1, 1)
        nc.scalar.activation(
            warm[:], warm[:], mybir.ActivationFunctionType.Exp, scale=1.0
        )

        # ---- Sync HWDGE queue: everything else in need-time order ----
        nc.sync.dma_start(w_t[:, 0 : 2 * WQK], w_d[:, 0 : 2 * WQK])
        nc.sync.dma_start(b_t[:], b_d[:])
        load_qk(nc.sync, k_in, kT_d, 0, 2)
        load_qk(nc.sync, q_in, qT_d, 2, 2)
        load_qk(nc.sync, k_in, kT_d, 2, 2)
        nc.sync.dma_start(
            w_t[:, 2 * WQK : 2 * WQK + WV + H], w_d[:, 2 * WQK : 2 * WQK + WV + H]
        )
        load_qk(nc.sync, k_in, kT_d, 4, 2)
        load_v(nc.sync, 0)
        load_v(nc.sync, 1)
        load_qk(nc.sync, k_in, kT_d, 6, 2)
        load_qk(nc.sync, q_in, qT_d, 4, 2)
        load_v(nc.sync, 2)
        load_v(nc.sync, 3)
        load_qk(nc.sync, q_in, qT_d, 6, 2)

        nc.vector.memset(vaug[:], 1.0)

        with tc.tile_pool(name="ps", bufs=3, space="PSUM") as ps, tc.tile_pool(
            name="op", bufs=1, space="PSUM"
        ) as op:
            # j0/j1 accumulate in the two halves; j2/j3 reuse them after
            # the j0/j1 normalizes have read the results out.
            po = op.tile([128, 1024], F32, tag="po")
            out_emitted = [0, 0, 0, 0]

            # ---- PE HAM warm-up on the weight tile while inputs stream ----
            for i in range(6):
                pw = ps.tile([128, 1024], F32, tag="ps")
                nc.tensor.matmul(
                    pw[:, 0:512],
                    lhsT=w_t[:, (i % 6) * 128 : (i % 6) * 128 + 128],
                    rhs=w_t[:, 0:512],
                    start=True,
                    stop=True,
                )

            def proj_chunk(x_t, w_sl, b_sl, dst, c0, w):
                # w s-columns starting at c0 of a q/k projection
                pj = ps.tile([128, 1024], F32, tag="ps")
                xc = x_cols(x_t, c0, w)
                for e in range(EC):
                    nc.tensor.matmul(
                        pj[:, 0:w],
                        lhsT=w_sl[:, e * 128 : (e + 1) * 128],
                        rhs=xc[:, :, e, :],
                        start=(e == 0),
                        stop=(e == EC - 1),
                    )
                nc.vector.tensor_scalar_add(dst[:, c0 : c0 + w], pj[:, 0:w], b_sl)

            def proj_v(c):
                # 512 s-columns of the v projection (single-width weights)
                pj = ps.tile([128, 1024], F32, tag="ps")
                xc = v_chunk(c)
                for e in range(EC):
                    nc.tensor.matmul(
                        pj[0:H, 0:512],
                        lhsT=wv_t[:, e * H : (e + 1) * H],
                        rhs=xc[:, e, :],
                        start=(e == 0),
                        stop=(e == EC - 1),
                    )
                nc.vector.tensor_scalar_add(
                    viT[:, c * 512 : (c + 1) * 512], pj[0:H, 0:512], bv_t
                )

            def tr_v(h):
                # viT [64, 8x128] -> vi [128, 8, 64] via PE transpose-mode,
                # then DVE-copy into the [vi | ones] vaug blocks
                tr = ps.tile([128, 512], F16, tag="ps")
                for i in range(8):
                    tb = h * 8 + i
                    nc.tensor.transpose(
                        tr[:, i * H : (i + 1) * H],
                        viT[:, tb * 128 : (tb + 1) * 128],
                        id_t,
                    )
                dst_ap = vaug[:, h * 1024 : (h + 1) * 1024].rearrange(
                    "p (t c) -> p t c", c=128
                )[:, :, 0:H]
                nc.vector.tensor_copy(
                    dst_ap, tr[:].rearrange("p (t k) -> p t k", k=H)
                )

            def unit(p, j):
                # scores + exp for t-blocks (2p, 2p+1) x q-cols [512j, 512j+512)
                u = ps.tile([128, 1024], F32, tag="ps")
                tbE, tbO = 2 * p, 2 * p + 1
                nc.tensor.matmul(
                    u[:, 0:512],
                    lhsT=kiT[0:H, tbE * 128 : (tbE + 1) * 128],
                    rhs=qiT[0:H, j * 512 : (j + 1) * 512],
                    start=True,
                    stop=True,
                )
                nc.tensor.matmul(
                    u[:, 512:1024],
                    lhsT=kiT[H:128, tbO * 128 : (tbO + 1) * 128],
                    rhs=qiT[H:128, j * 512 : (j + 1) * 512],
                    start=True,
                    stop=True,
                )
                nc.scalar.activation(
                    att[:, p * 4096 + j * 1024 : p * 4096 + (j + 1) * 1024],
                    u[:],
                    mybir.ActivationFunctionType.Exp,
                    scale=INV_SQRT_C,
                )

            def outs(tbs, js):
                for j in js:
                    pc = (j % 2) * 512
                    for tb in tbs:
                        base = (tb // 2) * 4096 + j * 1024 + (tb % 2) * 512
                        nc.tensor.matmul(
                            po[:, pc : pc + 512],
                            lhsT=vaug[:, tb * 128 : (tb + 1) * 128],
                            rhs=att[:, base : base + 512],
                            start=(tb == 0),
                            stop=(tb == TB - 1),
                        )
                        out_emitted[j] += 1
                    if out_emitted[j] == TB:
                        norm(j)

            def norm(j):
                jc = slice(j * 512, (j + 1) * 512)
                pc = slice((j % 2) * 512, (j % 2) * 512 + 512)
                nc.vector.tensor_copy(dsb[:, jc], po[H:128, pc])
                nc.vector.reciprocal_approx_fast(rec[:, jc], dsb[:, jc])
                nc.vector.tensor_tensor(
                    out_sb[:, jc], po[0:H, pc], rec[:, jc], op=mybir.AluOpType.mult
                )
                nc.sync.dma_start(outT_d[:, jc], out_sb[:, jc])

            # ---- PE stream: heavy work woven in <=2-unit slices ----
            proj_chunk(k_in, wk_t, bk_t, kiT, 0, 512)
            proj_chunk(q_in, wq_t, bq_t, qiT, 0, 256)
            proj_chunk(q_in, wq_t, bq_t, qiT, 256, 256)
            unit(0, 0)
            unit(1, 0)
            proj_chunk(q_in, wq_t, bq_t, qiT, 512, 512)
            unit(0, 1)
            unit(1, 1)
            proj_chunk(k_in, wk_t, bk_t, kiT, 512, 512)
            unit(2, 0)
            unit(3, 0)
            unit(2, 1)
            unit(3, 1)
            proj_chunk(k_in, wk_t, bk_t, kiT, 1024, 512)
            unit(4, 0)
            unit(5, 0)
            proj_v(0)
            unit(4, 1)
            unit(5, 1)
            proj_v(1)
            proj_chunk(k_in, wk_t, bk_t, kiT, 1536, 512)
            unit(6, 0)
            unit(7, 0)
            tr_v(0)
            unit(6, 1)
            unit(7, 1)
            outs(range(0, 2), (0, 1))
            outs(range(2, 4), (0, 1))
            projQ = proj_chunk
            projQ(q_in, wq_t, bq_t, qiT, 1024, 512)
            unit(0, 2)
            unit(1, 2)
            outs(range(4, 6), (0, 1))
            proj_v(2)
            unit(2, 2)
            unit(3, 2)
            outs(range(6, 8), (0, 1))
            proj_v(3)
            unit(4, 2)
            unit(5, 2)
            tr_v(1)
            unit(6, 2)
            unit(7, 2)
            outs(range(8, 10), (0, 1))
            outs(range(10, 12), (0, 1))
            projQ(q_in, wq_t, bq_t, qiT, 1536, 512)
            unit(0, 3)
            unit(1, 3)
            outs(range(12, 16), (0, 1))
            unit(2, 3)
            unit(3, 3)
            outs(range(0, 4), (2,))
            outs(range(4, 8), (2,))
            unit(4, 3)
            unit(5, 3)
            outs(range(8, 16), (2,))
            outs(range(0, 4), (3,))
            unit(6, 3)
            outs(range(4, 12), (3,))
            unit(7, 3)
            outs(range(12, 16), (3,))

    nc.compile()
    return nc


def _prep_inputs(q, k, v, Wq, bq, Wk, bk, Wv, bv):
    """Host-side layout prep: per-batch transpose + fp16 cast + re-chunk."""
    wq2 = np.concatenate([Wq, Wq], axis=1).astype(np.float16)  # [768, 128]
    wk2 = np.concatenate([Wk, Wk], axis=1).astype(np.float16)
    wv1 = np.asarray(Wv, dtype=np.float16)  # [768, 64]
    idp = np.zeros((128, H), dtype=np.float16)
    idp[0:H] = np.eye(H, dtype=np.float16)
    wpack = np.ascontiguousarray(
        np.concatenate(
            [
                wq2.reshape(EC, 128, 128).transpose(1, 0, 2).reshape(128, WQK),
                wk2.reshape(EC, 128, 128).transpose(1, 0, 2).reshape(128, WQK),
                wv1.reshape(EC, 128, H).transpose(1, 0, 2).reshape(128, WV),
                idp,
            ],
            axis=1,
        )
    )
    bpack = np.zeros((128, 4), dtype=np.float32)
    bpack[:, 0] = np.tile(np.asarray(bq, np.float32), 2)
    bpack[:, 1] = np.tile(np.asarray(bk, np.float32), 2)
    bpack[:, 2] = np.tile(np.asarray(bv, np.float32), 2)

    def chunk(xT, w):
        # [768, 2048] -> [(S/w)*128, 6*w]: s-chunk-major, e-chunks packed
        return np.ascontiguousarray(
            xT.reshape(EC, 128, S // w, w)
            .transpose(2, 1, 0, 3)
            .reshape((S // w) * 128, EC * w)
        )

    in_maps = []
    for i in range(B):
        qT = np.asarray(q[i], np.float16).T
        kT = np.asarray(k[i], np.float16).T
        vT = np.asarray(v[i], np.float16).T
        m = {
            "qTc": chunk(qT, CW),
            "kTc": chunk(kT, CW),
            "vTc": chunk(vT, 512),
            "wpack": wpack,
            "bpack": bpack,
        }
        in_maps.append(m)
    return in_maps


def run(trace=False, **inputs):
    """Build (cached), run on 8 cores, gather. Returns (out, BassKernelResults)."""
    if "nc" not in _CACHE:
        _CACHE["nc"] = build_program()
    nc = _CACHE["nc"]
    in_maps = _prep_inputs(**{k2: np.asarray(v2) for k2, v2 in inputs.items()})
    res = run_bass_kernel_spmd(nc, in_maps, list(range(B)), trace=trace)
    out = np.stack([np.ascontiguousarray(res.results[i]["outT"].T) for i in range(B)])
    return out.astype(np.float32), res


def kernel(**inputs) -> np.ndarray:
    out, _ = run(trace=False, **inputs)
    return out
